# revision 14
# baseline (speedup 1.0000x reference)
"""Trainium2 Bass kernel for nn_AxwinLowMixear (CSWin two-branch + global attention).

Sharding (8 cores): core = 2*b + role. Each core handles batch b:
  - CSWin branch `role` (96 output channels, all tokens, window-local order)
  - Global attention: slot0 = head (0 if role==0 else 2) full rows,
    slot1 = head 1 half rows (role0: rows 0:1568, role1: rows 1568:3136
    via a 1568-token rotation of its xa copy so the program is SPMD-uniform).

v4 design notes:
  - Softmax normalization is deferred to the host: the device emits
    numerators plus a denominator row (from an ones-column in V) and the
    host divides. The depthwise-conv LePE term is computed on the host
    from the device-produced v image (vt_out).
  - Attention probabilities (exp output) and V are fp8 e4m3; A@V runs in
    DoubleRow perf mode contracting two 128-j blocks per pass (2x PE).
  - exp range control: logits get a -BIAS shift folded into the QK matmul
    via zero-padded contraction rows (K bias row = 1, Q bias row =
    -BIAS/scale). A nonzero ACT bias would cost an extra SBUF const read
    (~185ns/instr), so the matmul fold is strictly cheaper.
  - All weights packed into one DRAM tensor (one DMA); xa loaded as
    [128, 3136] chunks with issues spread over the sync/gpsimd/scalar
    queues; output DMAs ride the gpsimd queue.
  - Jobs: the two leftover small query ranges merge into one 608-wide
    job -> 5 jobs, 125 global exp ACTs (one per (job, j-block)).
  - Only prep chunks 0-1 run before the first job; the remaining prep,
    V-prep and all cswin work stream in as fillers (4 per global
    iteration early on, 2 later) so the first exp starts ~10us in.
"""

import numpy as np
import ml_dtypes

B, DIM, RES, N = 4, 384, 56, 3136
TD, CSC = 192, 96
CS_SCALE = 48 ** -0.5
DN_SCALE = 64 ** -0.5
ROT = 1568
NJP = 3200          # global j padded (25 blocks of 128)
WPAD = 512          # cswin window j padded (4 blocks of 128)
NW = 8              # windows per image
WTOK = 392          # real tokens per window
VTW = 16 + NW * 448  # vt width: (56,8)-padded images + edge pads
BIAS = 2.2          # logit downshift for fp8 exp range
QB_G = -BIAS / DN_SCALE
QB_CS = -BIAS / CS_SCALE

BF = ml_dtypes.bfloat16
F8 = ml_dtypes.float8_e4m3

# (slot, qi0, qi1, psum col offset) segments per job
JOBS = [
    [(0, 0, 1024, 0)],
    [(0, 1024, 2048, 0)],
    [(0, 2048, 3072, 0)],
    [(1, 0, 1024, 0)],
    [(0, 3072, 3136, 0), (1, 1024, 1568, 64)],
]

# packed weight layout: name -> (col offset, n_chunks, cols per chunk)
WOFF = {}
_off = 0
for _nm, _ch, _c in [("wp2", 3, 256), ("wq_g0", 2, 128), ("wk_g0", 2, 128),
                     ("wq_g1", 2, 128), ("wk_g1", 2, 128), ("wv_g", 2, 130),
                     ("wp1", 3, 256), ("wq_cs", 2, 128), ("wk_cs0", 2, 128),
                     ("wk_cs1", 2, 128), ("wv_cs", 2, 128), ("wv_csT", 2, 128)]:
    WOFF[_nm] = (_off, _ch, _c)
    _off += _ch * _c
WCOLS = _off

_compiled = None


# ---------------------------------------------------------------- host prep --

def _cswin_perm(role):
    t = np.arange(N)
    w, rem = t // WTOK, t % WTOK
    r_, c_ = rem // 7, rem % 7
    if role == 0:
        return 56 * r_ + 7 * w + c_
    return 56 * (7 * w + c_) + r_


def _host_consts():
    """Ones-rows (1 on real tokens, 0 on pads) DMA'd into the spare
    channel row 64 of up[1]/dn[1]; together with bias/ones entries in
    weight row 192 they make the prep matmuls emit the Q/K logit-bias
    rows and the V ones-columns directly."""
    m = {}
    kcs_r = np.zeros((1, NW * WPAD), np.float32)
    kcs_r.reshape(NW, WPAD)[:, 0:WTOK] = 1.0
    m["onerow_cs"] = kcs_r.astype(BF)
    kg = np.zeros((1, NJP), np.float32)
    kg[0, 0:N] = 1.0
    m["onerow_g"] = kg.astype(BF)
    return m


def _host_inputs(inputs, core, consts):
    b, role = core // 2, core % 2
    xa = np.asarray(inputs["xa"], np.float32).reshape(B, DIM, N)[b]
    qkv_up = np.asarray(inputs["qkv_up_w"], np.float32)
    qkv_dn = np.asarray(inputs["qkv_dn_w"], np.float32)
    perm_cs = _cswin_perm(role)
    rot = 0 if role == 0 else ROT
    perm_rot = (np.arange(N) + rot) % N

    m = dict(consts)
    m["xa_cs"] = xa[:, perm_cs].astype(BF)
    m["xa_gl"] = xa[:, perm_rot].astype(BF)

    heads = (0, 1) if role == 0 else (2, 1)
    base = role * 96

    W = {}
    wp2 = np.zeros((384, 256), np.float32)
    wp2[:, :192] = np.asarray(inputs["proj2_w"], np.float32).T
    W["wp2"] = wp2
    wp1 = np.zeros((384, 256), np.float32)
    wp1[:, :192] = np.asarray(inputs["proj1_w"], np.float32).T
    W["wp1"] = wp1

    for s, h in enumerate(heads):
        a = np.zeros((256, 128), np.float32)
        a[0:192, 0:64] = qkv_dn[h * 64:(h + 1) * 64].T
        a[192, 64] = QB_G
        W[f"wq_g{s}"] = a
        a = np.zeros((256, 128), np.float32)
        a[0:192, 0:64] = qkv_dn[192 + h * 64:192 + (h + 1) * 64].T
        a[192, 64] = 1.0
        W[f"wk_g{s}"] = a
    wvg = np.zeros((256, 130), np.float32)
    wvg[:192, 0:64] = qkv_dn[384 + heads[0] * 64:384 + (heads[0] + 1) * 64].T
    wvg[:192, 65:129] = qkv_dn[384 + heads[1] * 64:384 + (heads[1] + 1) * 64].T
    wvg[192, 64] = 1.0
    wvg[192, 129] = 1.0
    W["wv_g"] = wvg

    wq = np.zeros((256, 128), np.float32)
    wq[:192, 0:48] = qkv_up[base:base + 48].T
    wq[:192, 64:112] = qkv_up[base + 48:base + 96].T
    wq[192, 48] = QB_CS
    W["wq_cs"] = wq
    wk0 = np.zeros((256, 128), np.float32)
    wk0[:192, 0:48] = qkv_up[192 + base:192 + base + 48].T
    wk0[192, 48] = 1.0
    W["wk_cs0"] = wk0
    wk1 = np.zeros((256, 128), np.float32)
    wk1[:192, 64:112] = qkv_up[192 + base + 48:192 + base + 96].T
    wk1[192, 48] = 1.0
    W["wk_cs1"] = wk1
    # cswin v weights: per-head block of 64 cols [v(48) | one | 0(15)]
    wv = np.zeros((256, 128), np.float32)
    wv[:192, 0:48] = qkv_up[384 + base:384 + base + 48].T
    wv[:192, 64:112] = qkv_up[384 + base + 48:384 + base + 96].T
    wv[192, 48] = 1.0
    wv[192, 112] = 1.0
    W["wv_cs"] = wv
    wvT = np.zeros((256, 128), np.float32)
    wvT[0:192, 0:96] = qkv_up[384 + base:384 + base + 96].T
    W["wv_csT"] = wvT

    pack = np.zeros((128, WCOLS), np.float32)
    for nm, (off, ch, c) in WOFF.items():
        for k in range(ch):
            src = W[nm][k * 128:(k + 1) * 128]
            pack[:src.shape[0], off + k * c:off + k * c + c] = src
    m["wpack"] = pack.astype(BF)
    return m


def _host_lepe(vt, role, inputs):
    """Depthwise 3x3 conv (+bias) over per-window (56,7) images, from the
    device-produced padded v image vt [96, VTW] (bf16)."""
    lw = np.asarray(inputs["lepe_w0" if role == 0 else "lepe_w1"], np.float32)[:, 0]
    lb = np.asarray(inputs["lepe_b0" if role == 0 else "lepe_b1"], np.float32)
    if role == 1:
        lw = lw.transpose(0, 2, 1)
    v = np.asarray(vt, np.float32)[:, 8:8 + NW * 448]
    v = v.reshape(96, NW, 56, 8)[:, :, :, 0:7]          # (C, w, r, c)
    vp = np.zeros((96, NW, 58, 9), np.float32)
    vp[:, :, 1:57, 1:8] = v
    out = np.zeros((96, NW, 56, 7), np.float32)
    for dr in range(3):
        for dc in range(3):
            out += lw[:, dr, dc][:, None, None, None] * \
                vp[:, :, dr:dr + 56, dc:dc + 7]
    out += lb[:, None, None, None]
    return out.reshape(96, N)                            # window-token order


def _assemble(results, inputs):
    out = np.zeros((B, DIM, N), np.float32)
    for core in range(8):
        b, role = core // 2, core % 2
        part = np.asarray(results[core]["out_part"], np.float32)
        lepe = _host_lepe(results[core]["vt_out"], role, inputs)
        perm_cs = _cswin_perm(role)
        rot = 0 if role == 0 else ROT
        base = role * 96
        for h in range(2):
            num = part[h * 49:h * 49 + 48]
            den = part[h * 49 + 48]
            lep = lepe[h * 48:(h + 1) * 48]
            out[b, base + h * 48:base + (h + 1) * 48, perm_cs] = \
                (num / den + lep).T
        h0 = 0 if role == 0 else 2
        g0 = part[98:162] / part[162]
        out[b, 192 + h0 * 64:192 + (h0 + 1) * 64] = np.roll(g0, rot, axis=1)
        g1 = part[163:227] / part[227]
        if role == 0:
            out[b, 256:320, 0:ROT] = g1[:, 0:ROT]
        else:
            out[b, 256:320, ROT:N] = g1[:, 0:ROT]
    return out.reshape(B, DIM, RES, RES).astype(np.float32)


# ---------------------------------------------------------------- bass build --

def _chunks(co, w):
    """Split psum column range [co, co+w) at 512 (bank) boundaries."""
    res = []
    u = co
    while u < co + w:
        nxt = min(co + w, (u // 512 + 1) * 512)
        res.append((u, nxt - u))
        u = nxt
    return res


def _build():
    import concourse.bacc as bacc
    import concourse.mybir as mybir
    import concourse.tile as tile
    import concourse.bass as bass

    fp32 = mybir.dt.float32
    bf16 = mybir.dt.bfloat16
    fp8 = mybir.dt.float8e4
    EXP = mybir.ActivationFunctionType.Exp
    DR = mybir.MatmulPerfMode.DoubleRow

    nc = bacc.Bacc("TRN2", target_bir_lowering=False, debug=False, num_devices=8)

    D = {}
    def din(name, shape, dt=None):
        D[name] = nc.dram_tensor(name, shape, dt or bf16, kind="ExternalInput")
    din("xa_cs", [DIM, N]); din("xa_gl", [DIM, N])
    din("wpack", [128, WCOLS])
    din("onerow_cs", [1, NW * WPAD]); din("onerow_g", [1, NJP])
    out_part = nc.dram_tensor("out_part", [228, N], fp32, kind="ExternalOutput")
    vt_out = nc.dram_tensor("vt_out", [96, VTW], bf16, kind="ExternalOutput")

    with tile.TileContext(nc) as tc:
        with (
            tc.tile_pool(name="w", bufs=1) as wp,
            tc.tile_pool(name="act", bufs=1) as ap,
            tc.tile_pool(name="outp", bufs=2) as op,
            tc.tile_pool(name="xap", bufs=1) as xap,
        ):
            # ---- input DMAs: critical bytes first (global weights + the
            # first halves of xa_gl), split small so the first prep matmul
            # isn't gated on whole-tensor transfers; 3 issue queues ----
            wtile = wp.tile([128, WCOLS], bf16, tag="wtile", name="wtile")
            WG = 2052  # cols of global weights (wp2..wv_g) at the front
            HN = 1568  # half of N

            def wslice(nm, c):
                off, ch, cols = WOFF[nm]
                return wtile[:, off + c * cols: off + (c + 1) * cols]

            xgl = [xap.tile([128, N], bf16, tag=f"xgl{c}", name=f"xgl{c}_t")
                   for c in range(3)]
            xcs = [xap.tile([128, N], bf16, tag=f"xcs{c}", name=f"xcs{c}_t")
                   for c in range(3)]
            Q_ = (nc.sync, nc.gpsimd, nc.scalar)
            # wave 1: global weights + xa_gl first halves
            nc.sync.dma_start(wtile[:, 0:WG], D["wpack"][:, 0:WG])
            for c in range(3):
                Q_[(c + 1) % 3].dma_start(
                    xgl[c][:, 0:HN], D["xa_gl"][c * 128:(c + 1) * 128, 0:HN])
            # wave 2: xa_gl second halves + cswin weights
            for c in range(3):
                Q_[(c + 1) % 3].dma_start(
                    xgl[c][:, HN:N], D["xa_gl"][c * 128:(c + 1) * 128, HN:N])
            nc.sync.dma_start(wtile[:, WG:WCOLS], D["wpack"][:, WG:WCOLS])
            # wave 3: xa_cs
            for c in range(3):
                Q_[(c + 1) % 3].dma_start(
                    xcs[c][:], D["xa_cs"][c * 128:(c + 1) * 128, :])

            # ---- persistent activation tiles ----
            qt_cs = ap.tile([128, N], bf16, tag="qt_cs", name="qt_cs")
            kcs = [ap.tile([128, NW * WPAD], bf16, tag=f"kcs{h}", name=f"kcs{h}")
                   for h in range(2)]
            vt_cs = ap.tile([128, VTW], bf16, tag="vt_cs", name="vt_cs")
            vcs = ap.tile([128, NW * 4 * 128], fp8, tag="vcs", name="vcs")
            Q = [ap.tile([128, N], bf16, tag=f"Q{s}", name=f"Q{s}") for s in range(2)]
            K = [ap.tile([128, NJP], bf16, tag=f"K{s}", name=f"K{s}") for s in range(2)]
            V = ap.tile([128, 13 * 320], fp8, tag="V", name="V")
            pt = [ap.tile([128, 2048], fp8, tag=f"ptg{p}", name=f"ptg{p}")
                  for p in range(13)]
            ptcs = [ap.tile([128, 800], fp8, tag=f"ptcs{g}", name=f"ptcs{g}")
                    for g in range(2)]
            up = [xap.tile([128, NW * WPAD], bf16, tag=f"up{i}", name=f"up{i}")
                  for i in range(2)]
            dn = [xap.tile([128, NJP], bf16, tag=f"dn{i}", name=f"dn{i}")
                  for i in range(2)]

            # pad-region fills (gpsimd; data regions are overwritten later)
            for h in range(2):
                nc.gpsimd.memset(
                    kcs[h][:].rearrange("p (w c) -> p w c", c=WPAD)[:, :, WTOK:WPAD], 0.0)
            nc.gpsimd.memset(vt_cs[:, 0:8], 0.0)
            nc.gpsimd.memset(vt_cs[:, VTW - 8:VTW], 0.0)
            nc.gpsimd.memset(
                vt_cs[:, 8:VTW - 8].rearrange("p (x c) -> p x c", c=8)[:, :, 7:8], 0.0)
            nc.gpsimd.memset(K[0][:, N:NJP], 0.0)
            nc.gpsimd.memset(K[1][:, N:NJP], 0.0)
            nc.gpsimd.memset(V[:, 12 * 320:13 * 320], 0.0)
            nc.gpsimd.memset(pt[12][:, 1024:2048], 0.0)
            nc.gpsimd.memset(
                up[0][:].rearrange("p (w c) -> p w c", c=WPAD)[:, :, WTOK:WPAD], 0.0)
            nc.gpsimd.memset(
                up[1][0:64, :].rearrange("p (w c) -> p w c", c=WPAD)[:, :, WTOK:WPAD], 0.0)
            nc.gpsimd.memset(up[1][64:128, :], 0.0)
            nc.gpsimd.memset(dn[0][:, N:NJP], 0.0)
            nc.gpsimd.memset(dn[1][0:64, N:NJP], 0.0)
            nc.gpsimd.memset(dn[1][64:128, :], 0.0)
            # ones rows (channel 192): the prep matmuls turn these into the
            # Q/K bias rows and V ones-columns via weight row 192
            nc.sync.dma_start(up[1][64:65, :], D["onerow_cs"][:])
            nc.sync.dma_start(dn[1][64:65, :], D["onerow_g"][:])

            with tc.tile_pool(name="pprep", bufs=2,
                              space=bass.MemorySpace.PSUM) as pp:
                def g_proj2(o, nch):
                    def go():
                        ps = pp.tile([128, 512], fp32, tag="fill", name="fill")
                        sl = slice(nch * 448, (nch + 1) * 448)
                        for c in range(3):
                            nc.tensor.matmul(
                                ps[:, 0:448], wslice("wp2", c)[:, o * 128:(o + 1) * 128],
                                xgl[c][:, sl], start=(c == 0), stop=(c == 2))
                        if o == 0:
                            nc.vector.tensor_copy(dn[0][:, sl], ps[:, 0:448])
                        else:
                            nc.vector.tensor_copy(dn[1][0:64, sl], ps[0:64, 0:448])
                    return go

                def g_qk(nm, dst, nch):
                    def go():
                        ps = pp.tile([128, 512], fp32, tag="fill", name="fill")
                        sl = slice(nch * 448, (nch + 1) * 448)
                        for c in range(2):
                            nc.tensor.matmul(
                                ps[:, 0:448], wslice(nm, c), dn[c][:, sl],
                                start=(c == 0), stop=(c == 1))
                        nc.vector.tensor_copy(dst[:, sl], ps[:, 0:448])
                    return go

                def g_v(jb):
                    def go():
                        ps = pp.tile([128, 512], fp32, tag="fill", name="fill")
                        sl = slice(jb * 128, (jb + 1) * 128)
                        for c in range(2):
                            nc.tensor.matmul(ps[:, 0:130], dn[c][:, sl],
                                             wslice("wv_g", c),
                                             start=(c == 0), stop=(c == 1))
                        vb = (jb // 2) * 320 + (jb % 2) * 80
                        nc.vector.tensor_copy(V[:, vb:vb + 65], ps[:, 0:65])
                        nc.vector.tensor_copy(V[:, vb + 160:vb + 225], ps[:, 65:130])
                    return go

                def prep_nch(nch):
                    return [g_proj2(0, nch), g_proj2(1, nch),
                            g_qk("wq_g0", Q[0], nch), g_qk("wk_g0", K[0], nch),
                            g_qk("wq_g1", Q[1], nch), g_qk("wk_g1", K[1], nch)]

                for nch in range(7):
                    for item in prep_nch(nch):
                        item()

                filler = []
                for jb in range(25):
                    filler.append(g_v(jb))

                # ---- cswin work, appended to the same filler stream ----
                def f_proj(o, w):
                    def go():
                        ps = pp.tile([128, 512], fp32, tag="fill", name="fill")
                        sl = slice(w * WTOK, (w + 1) * WTOK)
                        dsl = slice(w * WPAD, w * WPAD + WTOK)
                        for c in range(3):
                            nc.tensor.matmul(
                                ps[:, 0:WTOK], wslice("wp1", c)[:, o * 128:(o + 1) * 128],
                                xcs[c][:, sl], start=(c == 0), stop=(c == 2))
                        if o == 0:
                            nc.vector.tensor_copy(up[0][:, dsl], ps[:, 0:WTOK])
                        else:
                            nc.vector.tensor_copy(up[1][0:64, dsl], ps[0:64, 0:WTOK])
                    return go

                def f_qkv(nm, w, dst):
                    def go():
                        psl = slice(w * WPAD, w * WPAD + WTOK)
                        ps = pp.tile([128, 512], fp32, tag="fill", name="fill")
                        for c in range(2):
                            nc.tensor.matmul(ps[:, 0:WTOK], wslice(nm, c),
                                             up[c][:, psl], start=(c == 0), stop=(c == 1))
                        if nm == "wq_cs":
                            nc.vector.tensor_copy(
                                dst[:, w * WTOK:(w + 1) * WTOK], ps[:, 0:WTOK])
                        elif nm == "wv_csT":
                            vdst = vt_cs[:, 8 + w * 448:8 + (w + 1) * 448] \
                                .rearrange("p (r c) -> p r c", c=8)[:, :, 0:7]
                            nc.vector.tensor_copy(
                                vdst, ps[:, 0:WTOK].rearrange("p (r c) -> p r c", c=7))
                            nc.gpsimd.dma_start(
                                vt_out[:, 8 + w * 448:8 + (w + 1) * 448],
                                vt_cs[0:96, 8 + w * 448:8 + (w + 1) * 448])
                        else:
                            nc.vector.tensor_copy(dst[:, psl], ps[:, 0:WTOK])
                    return go

                def f_vcs(w, jb):
                    def go():
                        ps2 = pp.tile([128, 512], fp32, tag="fill", name="fill")
                        jsl = slice(w * WPAD + jb * 128, w * WPAD + (jb + 1) * 128)
                        for c in range(2):
                            nc.tensor.matmul(ps2[:, 0:128], up[c][:, jsl],
                                             wslice("wv_cs", c),
                                             start=(c == 0), stop=(c == 1))
                        blk0 = (w * 2 + jb // 2) * 2
                        # 49 cols: v(48) plus the ones column from weight
                        # row 192 x the up[1] ones-row
                        dst = vcs[:].rearrange("p (b c) -> p b c", c=128)[
                            :, blk0:blk0 + 2, (jb % 2) * 64:(jb % 2) * 64 + 49]
                        nc.vector.tensor_copy(
                            dst,
                            ps2[:, 0:128].rearrange("p (h c) -> p h c", c=64)[:, :, 0:49])
                    return go

                def f_attn_s(w, h, g):
                    def go():
                        wsl = slice(w * WTOK, (w + 1) * WTOK)
                        ps = pp.tile([128, 1024], fp32, tag="sg", name="sg")
                        for jj in range(2):
                            jb = g * 2 + jj
                            nc.tensor.matmul(
                                ps[:, jj * 512:jj * 512 + WTOK],
                                kcs[h][:, w * WPAD + jb * 128:w * WPAD + (jb + 1) * 128],
                                qt_cs[:, wsl])
                        nc.scalar.activation(
                            ptcs[g][:].rearrange("p (t c) -> p t c", c=400)[:, :, 0:WTOK],
                            ps[:].rearrange("p (t c) -> p t c", c=512)[:, :, 0:WTOK],
                            EXP, scale=CS_SCALE)
                    return go

                def f_attn_av(w, h):
                    def go():
                        wsl = slice(w * WTOK, (w + 1) * WTOK)
                        po = pp.tile([128, 512], fp32, tag="fill", name="fill")
                        for g in range(2):
                            blk = (w * 2 + g) * 2 + h
                            vp = vcs[:, blk * 128:(blk + 1) * 128] \
                                .rearrange("p (t c) -> p t c", t=2)[:, :, 0:49]
                            nc.tensor.matmul(
                                po[0:49, 0:WTOK], vp,
                                ptcs[g][:].rearrange("p (t n) -> p t n", t=2)[:, :, 0:WTOK],
                                start=(g == 0), stop=(g == 1), perf_mode=DR)
                        fin = op.tile([128, 512], fp32, tag="fin_cs", name="fin_cs")
                        nc.vector.tensor_copy(fin[0:49, 0:WTOK], po[0:49, 0:WTOK])
                        nc.gpsimd.dma_start(
                            out_part[h * 49:(h + 1) * 49, wsl], fin[0:49, 0:WTOK])
                    return go

                for w in range(NW):
                    filler.append(f_proj(0, w))
                    filler.append(f_proj(1, w))
                for w in range(NW):
                    filler.append(f_qkv("wq_cs", w, qt_cs))
                    filler.append(f_qkv("wk_cs0", w, kcs[0]))
                    filler.append(f_qkv("wk_cs1", w, kcs[1]))
                    filler.append(f_qkv("wv_csT", w, None))
                    for jb in range(4):
                        filler.append(f_vcs(w, jb))
                    for h in range(2):
                        filler.append(f_attn_s(w, h, 0))
                        filler.append(f_attn_s(w, h, 1))
                        filler.append(f_attn_av(w, h))

                # ---- global attention (software-pipelined, fp8 DoubleRow) ----
                def g_av(segs, ppo, p, start, stop):
                    for (s, i0, i1, co) in segs:
                        vpair = V[:, p * 320 + s * 160:p * 320 + s * 160 + 160] \
                            .rearrange("p (t c) -> p t c", t=2)[:, :, 0:65]
                        for (u, sw) in _chunks(co, i1 - i0):
                            nc.tensor.matmul(
                                ppo[u // 512][0:65, u % 512:u % 512 + sw], vpair,
                                pt[p][:].rearrange("p (t n) -> p t n", t=2)[:, :, u:u + sw],
                                start=start, stop=stop, perf_mode=DR)

                def g_out(segs, ppo):
                    for (s, i0, i1, co) in segs:
                        for (u, sw) in _chunks(co, i1 - i0):
                            on = op.tile([128, 512], fp32, tag="og_sb", name="og_sb")
                            nc.vector.tensor_copy(
                                on[0:65, 0:sw], ppo[u // 512][0:65, u % 512:u % 512 + sw])
                            nc.gpsimd.dma_start(
                                out_part[98 + s * 65:98 + s * 65 + 65,
                                         i0 + u - co:i0 + u - co + sw],
                                on[0:65, 0:sw])

                giter = 0
                for job in JOBS:
                    Wdt = max(co + (i1 - i0) for (s, i0, i1, co) in job)
                    po_subs = [pp.tile([128, 512], fp32, tag="og", name="og")
                               for _ in range((Wdt + 511) // 512)]
                    for p in range(13):
                        for jj in (2 * p, 2 * p + 1):
                            if jj >= 25:
                                continue
                            ps = pp.tile([128, 1024], fp32, tag="sg", name="sg")
                            for (s, i0, i1, co) in job:
                                for (u, sw) in _chunks(co, i1 - i0):
                                    nc.tensor.matmul(
                                        ps[:, u:u + sw],
                                        K[s][:, jj * 128:(jj + 1) * 128],
                                        Q[s][:, i0 + u - co:i0 + u - co + sw])
                            nc.scalar.activation(
                                pt[p][:, (jj % 2) * 1024:(jj % 2) * 1024 + Wdt],
                                ps[:, 0:Wdt], EXP, scale=DN_SCALE)
                        # consume this job's own pt with one-iteration lag so
                        # A@V overlaps the exp stream instead of tailing it
                        if p >= 1:
                            g_av(job, po_subs, p - 1, p == 1, False)
                        npop = 4 if giter < 16 else 2
                        for _ in range(npop):
                            if filler:
                                filler.pop(0)()
                        giter += 1
                    g_av(job, po_subs, 12, False, True)
                    g_out(job, po_subs)
                while filler:
                    filler.pop(0)()

    nc.compile()
    return nc


def kernel(**inputs) -> np.ndarray:
    global _compiled
    from concourse.bass_utils import run_bass_kernel_spmd
    if _compiled is None:
        _compiled = _build()
    nc = _compiled
    consts = _host_consts()
    in_maps = [_host_inputs(inputs, core, consts) for core in range(8)]
    res = run_bass_kernel_spmd(nc, in_maps, list(range(8)))
    return _assemble(res.results, inputs)


# revision 15
# speedup vs baseline: 1.0082x; 1.0082x over previous
"""Trainium2 Bass kernel for nn_AxwinLowMixear (CSWin two-branch + global attention).

Sharding (8 cores): core = 2*b + role. Each core handles batch b:
  - CSWin branch `role` (96 output channels, all tokens, window-local order)
  - Global attention: slot0 = head (0 if role==0 else 2) full rows,
    slot1 = head 1 half rows (role0: rows 0:1568, role1: rows 1568:3136
    via a 1568-token rotation of its xa copy so the program is SPMD-uniform).

v4 design notes:
  - Softmax normalization is deferred to the host: the device emits
    numerators plus a denominator row (from an ones-column in V) and the
    host divides. The depthwise-conv LePE term is computed on the host
    from the device-produced v image (vt_out).
  - Attention probabilities (exp output) and V are fp8 e4m3; A@V runs in
    DoubleRow perf mode contracting two 128-j blocks per pass (2x PE).
  - exp range control: logits get a -BIAS shift folded into the QK matmul
    via zero-padded contraction rows (K bias row = 1, Q bias row =
    -BIAS/scale). A nonzero ACT bias would cost an extra SBUF const read
    (~185ns/instr), so the matmul fold is strictly cheaper.
  - All weights packed into one DRAM tensor (one DMA); xa loaded as
    [128, 3136] chunks with issues spread over the sync/gpsimd/scalar
    queues; output DMAs ride the gpsimd queue.
  - Jobs: the two leftover small query ranges merge into one 608-wide
    job -> 5 jobs, 125 global exp ACTs (one per (job, j-block)).
  - Only prep chunks 0-1 run before the first job; the remaining prep,
    V-prep and all cswin work stream in as fillers (4 per global
    iteration early on, 2 later) so the first exp starts ~10us in.
"""

import numpy as np
import ml_dtypes

B, DIM, RES, N = 4, 384, 56, 3136
TD, CSC = 192, 96
CS_SCALE = 48 ** -0.5
DN_SCALE = 64 ** -0.5
ROT = 1568
NJP = 3200          # global j padded (25 blocks of 128)
WPAD = 512          # cswin window j padded (4 blocks of 128)
NW = 8              # windows per image
WTOK = 392          # real tokens per window
VTW = 16 + NW * 448  # vt width: (56,8)-padded images + edge pads
BIAS = 2.2          # logit downshift for fp8 exp range
QB_G = -BIAS / DN_SCALE
QB_CS = -BIAS / CS_SCALE

BF = ml_dtypes.bfloat16
F8 = ml_dtypes.float8_e4m3

# (slot, qi0, qi1, psum col offset) segments per job
JOBS = [
    [(0, 0, 1024, 0)],
    [(0, 1024, 2048, 0)],
    [(0, 2048, 3072, 0)],
    [(1, 0, 1024, 0)],
    [(0, 3072, 3136, 0), (1, 1024, 1568, 64)],
]

# packed weight layout: name -> (col offset, n_chunks, cols per chunk)
WOFF = {}
_off = 0
for _nm, _ch, _c in [("wp2", 3, 256), ("wq_g0", 2, 128), ("wk_g0", 2, 128),
                     ("wq_g1", 2, 128), ("wk_g1", 2, 128), ("wv_g", 2, 130),
                     ("wp1", 3, 256), ("wq_cs", 2, 128), ("wk_cs0", 2, 128),
                     ("wk_cs1", 2, 128), ("wv_cs", 2, 128), ("wv_csT", 2, 128)]:
    WOFF[_nm] = (_off, _ch, _c)
    _off += _ch * _c
WCOLS = _off

_compiled = None


# ---------------------------------------------------------------- host prep --

def _cswin_perm(role):
    t = np.arange(N)
    w, rem = t // WTOK, t % WTOK
    r_, c_ = rem // 7, rem % 7
    if role == 0:
        return 56 * r_ + 7 * w + c_
    return 56 * (7 * w + c_) + r_


def _host_consts():
    """Ones-rows (1 on real tokens, 0 on pads) DMA'd into the spare
    channel row 64 of up[1]/dn[1]; together with bias/ones entries in
    weight row 192 they make the prep matmuls emit the Q/K logit-bias
    rows and the V ones-columns directly."""
    m = {}
    kcs_r = np.zeros((1, NW * WPAD), np.float32)
    kcs_r.reshape(NW, WPAD)[:, 0:WTOK] = 1.0
    m["onerow_cs"] = kcs_r.astype(BF)
    kg = np.zeros((1, NJP), np.float32)
    kg[0, 0:N] = 1.0
    m["onerow_g"] = kg.astype(BF)
    return m


def _host_inputs(inputs, core, consts):
    b, role = core // 2, core % 2
    xa = np.asarray(inputs["xa"], np.float32).reshape(B, DIM, N)[b]
    qkv_up = np.asarray(inputs["qkv_up_w"], np.float32)
    qkv_dn = np.asarray(inputs["qkv_dn_w"], np.float32)
    perm_cs = _cswin_perm(role)
    rot = 0 if role == 0 else ROT
    perm_rot = (np.arange(N) + rot) % N

    m = dict(consts)
    m["xa_cs"] = xa[:, perm_cs].astype(BF)
    m["xa_gl"] = xa[:, perm_rot].astype(BF)

    heads = (0, 1) if role == 0 else (2, 1)
    base = role * 96

    W = {}
    wp2 = np.zeros((384, 256), np.float32)
    wp2[:, :192] = np.asarray(inputs["proj2_w"], np.float32).T
    W["wp2"] = wp2
    wp1 = np.zeros((384, 256), np.float32)
    wp1[:, :192] = np.asarray(inputs["proj1_w"], np.float32).T
    W["wp1"] = wp1

    for s, h in enumerate(heads):
        a = np.zeros((256, 128), np.float32)
        a[0:192, 0:64] = qkv_dn[h * 64:(h + 1) * 64].T
        a[192, 64] = QB_G
        W[f"wq_g{s}"] = a
        a = np.zeros((256, 128), np.float32)
        a[0:192, 0:64] = qkv_dn[192 + h * 64:192 + (h + 1) * 64].T
        a[192, 64] = 1.0
        W[f"wk_g{s}"] = a
    wvg = np.zeros((256, 130), np.float32)
    wvg[:192, 0:64] = qkv_dn[384 + heads[0] * 64:384 + (heads[0] + 1) * 64].T
    wvg[:192, 65:129] = qkv_dn[384 + heads[1] * 64:384 + (heads[1] + 1) * 64].T
    wvg[192, 64] = 1.0
    wvg[192, 129] = 1.0
    W["wv_g"] = wvg

    wq = np.zeros((256, 128), np.float32)
    wq[:192, 0:48] = qkv_up[base:base + 48].T
    wq[:192, 64:112] = qkv_up[base + 48:base + 96].T
    wq[192, 48] = QB_CS
    W["wq_cs"] = wq
    wk0 = np.zeros((256, 128), np.float32)
    wk0[:192, 0:48] = qkv_up[192 + base:192 + base + 48].T
    wk0[192, 48] = 1.0
    W["wk_cs0"] = wk0
    wk1 = np.zeros((256, 128), np.float32)
    wk1[:192, 64:112] = qkv_up[192 + base + 48:192 + base + 96].T
    wk1[192, 48] = 1.0
    W["wk_cs1"] = wk1
    # cswin v weights: per-head block of 64 cols [v(48) | one | 0(15)]
    wv = np.zeros((256, 128), np.float32)
    wv[:192, 0:48] = qkv_up[384 + base:384 + base + 48].T
    wv[:192, 64:112] = qkv_up[384 + base + 48:384 + base + 96].T
    wv[192, 48] = 1.0
    wv[192, 112] = 1.0
    W["wv_cs"] = wv
    wvT = np.zeros((256, 128), np.float32)
    wvT[0:192, 0:96] = qkv_up[384 + base:384 + base + 96].T
    W["wv_csT"] = wvT

    pack = np.zeros((128, WCOLS), np.float32)
    for nm, (off, ch, c) in WOFF.items():
        for k in range(ch):
            src = W[nm][k * 128:(k + 1) * 128]
            pack[:src.shape[0], off + k * c:off + k * c + c] = src
    m["wpack"] = pack.astype(BF)
    return m


def _host_lepe(vt, role, inputs):
    """Depthwise 3x3 conv (+bias) over per-window (56,7) images, from the
    device-produced padded v image vt [96, VTW] (bf16)."""
    lw = np.asarray(inputs["lepe_w0" if role == 0 else "lepe_w1"], np.float32)[:, 0]
    lb = np.asarray(inputs["lepe_b0" if role == 0 else "lepe_b1"], np.float32)
    if role == 1:
        lw = lw.transpose(0, 2, 1)
    v = np.asarray(vt, np.float32)[:, 8:8 + NW * 448]
    v = v.reshape(96, NW, 56, 8)[:, :, :, 0:7]          # (C, w, r, c)
    vp = np.zeros((96, NW, 58, 9), np.float32)
    vp[:, :, 1:57, 1:8] = v
    out = np.zeros((96, NW, 56, 7), np.float32)
    for dr in range(3):
        for dc in range(3):
            out += lw[:, dr, dc][:, None, None, None] * \
                vp[:, :, dr:dr + 56, dc:dc + 7]
    out += lb[:, None, None, None]
    return out.reshape(96, N)                            # window-token order


def _assemble(results, inputs):
    out = np.zeros((B, DIM, N), np.float32)
    for core in range(8):
        b, role = core // 2, core % 2
        part = np.asarray(results[core]["out_part"], np.float32)
        lepe = _host_lepe(results[core]["vt_out"], role, inputs)
        perm_cs = _cswin_perm(role)
        rot = 0 if role == 0 else ROT
        base = role * 96
        for h in range(2):
            num = part[h * 49:h * 49 + 48]
            den = part[h * 49 + 48]
            lep = lepe[h * 48:(h + 1) * 48]
            out[b, base + h * 48:base + (h + 1) * 48, perm_cs] = \
                (num / den + lep).T
        h0 = 0 if role == 0 else 2
        g0 = part[98:162] / part[162]
        out[b, 192 + h0 * 64:192 + (h0 + 1) * 64] = np.roll(g0, rot, axis=1)
        g1 = part[163:227] / part[227]
        if role == 0:
            out[b, 256:320, 0:ROT] = g1[:, 0:ROT]
        else:
            out[b, 256:320, ROT:N] = g1[:, 0:ROT]
    return out.reshape(B, DIM, RES, RES).astype(np.float32)


# ---------------------------------------------------------------- bass build --

def _chunks(co, w):
    """Split psum column range [co, co+w) at 512 (bank) boundaries."""
    res = []
    u = co
    while u < co + w:
        nxt = min(co + w, (u // 512 + 1) * 512)
        res.append((u, nxt - u))
        u = nxt
    return res


def _build():
    import concourse.bacc as bacc
    import concourse.mybir as mybir
    import concourse.tile as tile
    import concourse.bass as bass

    fp32 = mybir.dt.float32
    bf16 = mybir.dt.bfloat16
    fp8 = mybir.dt.float8e4
    EXP = mybir.ActivationFunctionType.Exp
    DR = mybir.MatmulPerfMode.DoubleRow

    nc = bacc.Bacc("TRN2", target_bir_lowering=False, debug=False, num_devices=8)

    D = {}
    def din(name, shape, dt=None):
        D[name] = nc.dram_tensor(name, shape, dt or bf16, kind="ExternalInput")
    din("xa_cs", [DIM, N]); din("xa_gl", [DIM, N])
    din("wpack", [128, WCOLS])
    din("onerow_cs", [1, NW * WPAD]); din("onerow_g", [1, NJP])
    out_part = nc.dram_tensor("out_part", [228, N], fp32, kind="ExternalOutput")
    vt_out = nc.dram_tensor("vt_out", [96, VTW], bf16, kind="ExternalOutput")

    with tile.TileContext(nc) as tc:
        with (
            tc.tile_pool(name="w", bufs=1) as wp,
            tc.tile_pool(name="act", bufs=1) as ap,
            tc.tile_pool(name="outp", bufs=2) as op,
            tc.tile_pool(name="xap", bufs=1) as xap,
        ):
            # ---- input DMAs: critical bytes first (global weights + the
            # first halves of xa_gl), split small so the first prep matmul
            # isn't gated on whole-tensor transfers; 3 issue queues ----
            wtile = wp.tile([128, WCOLS], bf16, tag="wtile", name="wtile")
            WG = 2052  # cols of global weights (wp2..wv_g) at the front
            HN = 1568  # half of N

            def wslice(nm, c):
                off, ch, cols = WOFF[nm]
                return wtile[:, off + c * cols: off + (c + 1) * cols]

            xgl = [xap.tile([128, N], bf16, tag=f"xgl{c}", name=f"xgl{c}_t")
                   for c in range(3)]
            xcs = [xap.tile([128, N], bf16, tag=f"xcs{c}", name=f"xcs{c}_t")
                   for c in range(3)]
            Q_ = (nc.sync, nc.gpsimd, nc.scalar)
            # wave 1: global weights + xa_gl first halves
            nc.sync.dma_start(wtile[:, 0:WG], D["wpack"][:, 0:WG])
            for c in range(3):
                Q_[(c + 1) % 3].dma_start(
                    xgl[c][:, 0:HN], D["xa_gl"][c * 128:(c + 1) * 128, 0:HN])
            # wave 2: xa_gl second halves + cswin weights
            for c in range(3):
                Q_[(c + 1) % 3].dma_start(
                    xgl[c][:, HN:N], D["xa_gl"][c * 128:(c + 1) * 128, HN:N])
            nc.sync.dma_start(wtile[:, WG:WCOLS], D["wpack"][:, WG:WCOLS])
            # wave 3: xa_cs
            for c in range(3):
                Q_[(c + 1) % 3].dma_start(
                    xcs[c][:], D["xa_cs"][c * 128:(c + 1) * 128, :])

            # ---- persistent activation tiles ----
            qt_cs = ap.tile([128, N], bf16, tag="qt_cs", name="qt_cs")
            kcs = [ap.tile([128, NW * WPAD], bf16, tag=f"kcs{h}", name=f"kcs{h}")
                   for h in range(2)]
            vt_cs = ap.tile([128, VTW], bf16, tag="vt_cs", name="vt_cs")
            vcs = ap.tile([128, NW * 4 * 128], fp8, tag="vcs", name="vcs")
            Q = [ap.tile([128, N], bf16, tag=f"Q{s}", name=f"Q{s}") for s in range(2)]
            K = [ap.tile([128, NJP], bf16, tag=f"K{s}", name=f"K{s}") for s in range(2)]
            V = ap.tile([128, 13 * 320], fp8, tag="V", name="V")
            pt = [ap.tile([128, 2048], fp8, tag=f"ptg{p}", name=f"ptg{p}")
                  for p in range(13)]
            ptcs = [ap.tile([128, 800], fp8, tag=f"ptcs{g}", name=f"ptcs{g}")
                    for g in range(2)]
            up = [xap.tile([128, NW * WPAD], bf16, tag=f"up{i}", name=f"up{i}")
                  for i in range(2)]
            dn = [xap.tile([128, NJP], bf16, tag=f"dn{i}", name=f"dn{i}")
                  for i in range(2)]

            # pad-region fills (gpsimd; data regions are overwritten later)
            for h in range(2):
                nc.gpsimd.memset(
                    kcs[h][:].rearrange("p (w c) -> p w c", c=WPAD)[:, :, WTOK:WPAD], 0.0)
            nc.gpsimd.memset(vt_cs[:, 0:8], 0.0)
            nc.gpsimd.memset(vt_cs[:, VTW - 8:VTW], 0.0)
            nc.gpsimd.memset(
                vt_cs[:, 8:VTW - 8].rearrange("p (x c) -> p x c", c=8)[:, :, 7:8], 0.0)
            nc.gpsimd.memset(K[0][:, N:NJP], 0.0)
            nc.gpsimd.memset(K[1][:, N:NJP], 0.0)
            nc.gpsimd.memset(V[:, 12 * 320:13 * 320], 0.0)
            nc.gpsimd.memset(pt[12][:, 1024:2048], 0.0)
            nc.gpsimd.memset(
                up[0][:].rearrange("p (w c) -> p w c", c=WPAD)[:, :, WTOK:WPAD], 0.0)
            nc.gpsimd.memset(
                up[1][0:64, :].rearrange("p (w c) -> p w c", c=WPAD)[:, :, WTOK:WPAD], 0.0)
            nc.gpsimd.memset(up[1][64:128, :], 0.0)
            nc.gpsimd.memset(dn[0][:, N:NJP], 0.0)
            nc.gpsimd.memset(dn[1][0:64, N:NJP], 0.0)
            nc.gpsimd.memset(dn[1][64:128, :], 0.0)
            # ones rows (channel 192): the prep matmuls turn these into the
            # Q/K bias rows and V ones-columns via weight row 192
            nc.sync.dma_start(up[1][64:65, :], D["onerow_cs"][:])
            nc.sync.dma_start(dn[1][64:65, :], D["onerow_g"][:])

            with tc.tile_pool(name="pprep", bufs=2,
                              space=bass.MemorySpace.PSUM) as pp:
                def g_proj2(o, nch):
                    def go():
                        ps = pp.tile([128, 512], fp32, tag="fill", name="fill")
                        sl = slice(nch * 448, (nch + 1) * 448)
                        for c in range(3):
                            nc.tensor.matmul(
                                ps[:, 0:448], wslice("wp2", c)[:, o * 128:(o + 1) * 128],
                                xgl[c][:, sl], start=(c == 0), stop=(c == 2))
                        if o == 0:
                            nc.vector.tensor_copy(dn[0][:, sl], ps[:, 0:448])
                        else:
                            nc.vector.tensor_copy(dn[1][0:64, sl], ps[0:64, 0:448])
                    return go

                def g_qk(nm, dst, nch):
                    def go():
                        ps = pp.tile([128, 512], fp32, tag="fill", name="fill")
                        sl = slice(nch * 448, (nch + 1) * 448)
                        for c in range(2):
                            nc.tensor.matmul(
                                ps[:, 0:448], wslice(nm, c), dn[c][:, sl],
                                start=(c == 0), stop=(c == 1))
                        nc.vector.tensor_copy(dst[:, sl], ps[:, 0:448])
                    return go

                def g_v(jb):
                    def go():
                        ps = pp.tile([128, 512], fp32, tag="fill", name="fill")
                        sl = slice(jb * 128, (jb + 1) * 128)
                        for c in range(2):
                            nc.tensor.matmul(ps[:, 0:130], dn[c][:, sl],
                                             wslice("wv_g", c),
                                             start=(c == 0), stop=(c == 1))
                        vb = (jb // 2) * 320 + (jb % 2) * 80
                        nc.vector.tensor_copy(V[:, vb:vb + 65], ps[:, 0:65])
                        nc.vector.tensor_copy(V[:, vb + 160:vb + 225], ps[:, 65:130])
                    return go

                def prep_nch(nch):
                    return [g_proj2(0, nch), g_proj2(1, nch),
                            g_qk("wq_g0", Q[0], nch), g_qk("wk_g0", K[0], nch),
                            g_qk("wq_g1", Q[1], nch), g_qk("wk_g1", K[1], nch)]

                # prep chunks 0-3 run before the first job (covers K/Q cols
                # 0:1792); chunks 4-6 and V-prep stream in as fillers whose
                # emission order is deadline-checked against consumers
                for nch in range(4):
                    for item in prep_nch(nch):
                        item()

                filler = []
                for jb in range(6):
                    filler.append(g_v(jb))
                filler.extend(prep_nch(4))
                for jb in range(6, 12):
                    filler.append(g_v(jb))
                filler.extend(prep_nch(5))
                for jb in range(12, 18):
                    filler.append(g_v(jb))
                filler.extend(prep_nch(6))
                for jb in range(18, 25):
                    filler.append(g_v(jb))

                # ---- cswin work, appended to the same filler stream ----
                def f_proj(o, w):
                    def go():
                        ps = pp.tile([128, 512], fp32, tag="fill", name="fill")
                        sl = slice(w * WTOK, (w + 1) * WTOK)
                        dsl = slice(w * WPAD, w * WPAD + WTOK)
                        for c in range(3):
                            nc.tensor.matmul(
                                ps[:, 0:WTOK], wslice("wp1", c)[:, o * 128:(o + 1) * 128],
                                xcs[c][:, sl], start=(c == 0), stop=(c == 2))
                        if o == 0:
                            nc.vector.tensor_copy(up[0][:, dsl], ps[:, 0:WTOK])
                        else:
                            nc.vector.tensor_copy(up[1][0:64, dsl], ps[0:64, 0:WTOK])
                    return go

                def f_qkv(nm, w, dst):
                    def go():
                        psl = slice(w * WPAD, w * WPAD + WTOK)
                        ps = pp.tile([128, 512], fp32, tag="fill", name="fill")
                        for c in range(2):
                            nc.tensor.matmul(ps[:, 0:WTOK], wslice(nm, c),
                                             up[c][:, psl], start=(c == 0), stop=(c == 1))
                        if nm == "wq_cs":
                            nc.vector.tensor_copy(
                                dst[:, w * WTOK:(w + 1) * WTOK], ps[:, 0:WTOK])
                        elif nm == "wv_csT":
                            vdst = vt_cs[:, 8 + w * 448:8 + (w + 1) * 448] \
                                .rearrange("p (r c) -> p r c", c=8)[:, :, 0:7]
                            nc.vector.tensor_copy(
                                vdst, ps[:, 0:WTOK].rearrange("p (r c) -> p r c", c=7))
                            nc.gpsimd.dma_start(
                                vt_out[:, 8 + w * 448:8 + (w + 1) * 448],
                                vt_cs[0:96, 8 + w * 448:8 + (w + 1) * 448])
                        else:
                            nc.vector.tensor_copy(dst[:, psl], ps[:, 0:WTOK])
                    return go

                def f_vcs(w, jb):
                    def go():
                        ps2 = pp.tile([128, 512], fp32, tag="fill", name="fill")
                        jsl = slice(w * WPAD + jb * 128, w * WPAD + (jb + 1) * 128)
                        for c in range(2):
                            nc.tensor.matmul(ps2[:, 0:128], up[c][:, jsl],
                                             wslice("wv_cs", c),
                                             start=(c == 0), stop=(c == 1))
                        blk0 = (w * 2 + jb // 2) * 2
                        # 49 cols: v(48) plus the ones column from weight
                        # row 192 x the up[1] ones-row
                        dst = vcs[:].rearrange("p (b c) -> p b c", c=128)[
                            :, blk0:blk0 + 2, (jb % 2) * 64:(jb % 2) * 64 + 49]
                        nc.vector.tensor_copy(
                            dst,
                            ps2[:, 0:128].rearrange("p (h c) -> p h c", c=64)[:, :, 0:49])
                    return go

                def f_attn_s(w, h, g):
                    def go():
                        wsl = slice(w * WTOK, (w + 1) * WTOK)
                        ps = pp.tile([128, 1024], fp32, tag="sg", name="sg")
                        for jj in range(2):
                            jb = g * 2 + jj
                            nc.tensor.matmul(
                                ps[:, jj * 512:jj * 512 + WTOK],
                                kcs[h][:, w * WPAD + jb * 128:w * WPAD + (jb + 1) * 128],
                                qt_cs[:, wsl])
                        nc.scalar.activation(
                            ptcs[g][:].rearrange("p (t c) -> p t c", c=400)[:, :, 0:WTOK],
                            ps[:].rearrange("p (t c) -> p t c", c=512)[:, :, 0:WTOK],
                            EXP, scale=CS_SCALE)
                    return go

                def f_attn_av(w, h):
                    def go():
                        wsl = slice(w * WTOK, (w + 1) * WTOK)
                        po = pp.tile([128, 512], fp32, tag="fill", name="fill")
                        for g in range(2):
                            blk = (w * 2 + g) * 2 + h
                            vp = vcs[:, blk * 128:(blk + 1) * 128] \
                                .rearrange("p (t c) -> p t c", t=2)[:, :, 0:49]
                            nc.tensor.matmul(
                                po[0:49, 0:WTOK], vp,
                                ptcs[g][:].rearrange("p (t n) -> p t n", t=2)[:, :, 0:WTOK],
                                start=(g == 0), stop=(g == 1), perf_mode=DR)
                        fin = op.tile([128, 512], fp32, tag="fin_cs", name="fin_cs")
                        nc.vector.tensor_copy(fin[0:49, 0:WTOK], po[0:49, 0:WTOK])
                        nc.gpsimd.dma_start(
                            out_part[h * 49:(h + 1) * 49, wsl], fin[0:49, 0:WTOK])
                    return go

                for w in range(NW):
                    filler.append(f_proj(0, w))
                    filler.append(f_proj(1, w))
                for w in range(NW):
                    filler.append(f_qkv("wq_cs", w, qt_cs))
                    filler.append(f_qkv("wk_cs0", w, kcs[0]))
                    filler.append(f_qkv("wk_cs1", w, kcs[1]))
                    filler.append(f_qkv("wv_csT", w, None))
                for w in range(NW):
                    for jb in range(4):
                        filler.append(f_vcs(w, jb))
                for w in range(NW):
                    for h in range(2):
                        filler.append(f_attn_s(w, h, 0))
                        filler.append(f_attn_s(w, h, 1))
                        filler.append(f_attn_av(w, h))

                # ---- global attention (software-pipelined, fp8 DoubleRow) ----
                def g_av(segs, ppo, p, start, stop):
                    for (s, i0, i1, co) in segs:
                        vpair = V[:, p * 320 + s * 160:p * 320 + s * 160 + 160] \
                            .rearrange("p (t c) -> p t c", t=2)[:, :, 0:65]
                        for (u, sw) in _chunks(co, i1 - i0):
                            nc.tensor.matmul(
                                ppo[u // 512][0:65, u % 512:u % 512 + sw], vpair,
                                pt[p][:].rearrange("p (t n) -> p t n", t=2)[:, :, u:u + sw],
                                start=start, stop=stop, perf_mode=DR)

                def g_out(segs, ppo):
                    for (s, i0, i1, co) in segs:
                        for (u, sw) in _chunks(co, i1 - i0):
                            on = op.tile([128, 512], fp32, tag="og_sb", name="og_sb")
                            nc.vector.tensor_copy(
                                on[0:65, 0:sw], ppo[u // 512][0:65, u % 512:u % 512 + sw])
                            nc.gpsimd.dma_start(
                                out_part[98 + s * 65:98 + s * 65 + 65,
                                         i0 + u - co:i0 + u - co + sw],
                                on[0:65, 0:sw])

                giter = 0
                for job in JOBS:
                    Wdt = max(co + (i1 - i0) for (s, i0, i1, co) in job)
                    po_subs = [pp.tile([128, 512], fp32, tag="og", name="og")
                               for _ in range((Wdt + 511) // 512)]
                    for p in range(13):
                        for jj in (2 * p, 2 * p + 1):
                            if jj >= 25:
                                continue
                            ps = pp.tile([128, 1024], fp32, tag="sg", name="sg")
                            for (s, i0, i1, co) in job:
                                for (u, sw) in _chunks(co, i1 - i0):
                                    nc.tensor.matmul(
                                        ps[:, u:u + sw],
                                        K[s][:, jj * 128:(jj + 1) * 128],
                                        Q[s][:, i0 + u - co:i0 + u - co + sw])
                            nc.scalar.activation(
                                pt[p][:, (jj % 2) * 1024:(jj % 2) * 1024 + Wdt],
                                ps[:, 0:Wdt], EXP, scale=DN_SCALE)
                        # consume this job's own pt with one-iteration lag so
                        # A@V overlaps the exp stream instead of tailing it
                        if p >= 1:
                            g_av(job, po_subs, p - 1, p == 1, False)
                        npop = 4 if giter < 16 else (3 if giter < 30 else 2)
                        for _ in range(npop):
                            if filler:
                                filler.pop(0)()
                        giter += 1
                    g_av(job, po_subs, 12, False, True)
                    g_out(job, po_subs)
                while filler:
                    filler.pop(0)()

    nc.compile()
    return nc


def kernel(**inputs) -> np.ndarray:
    global _compiled
    from concourse.bass_utils import run_bass_kernel_spmd
    if _compiled is None:
        _compiled = _build()
    nc = _compiled
    consts = _host_consts()
    in_maps = [_host_inputs(inputs, core, consts) for core in range(8)]
    res = run_bass_kernel_spmd(nc, in_maps, list(range(8)))
    return _assemble(res.results, inputs)
